# revision 7
# baseline (speedup 1.0000x reference)
"""Single-head causal self-attention (B=256, T=320, C=1024, H=64) on 8 trn2 NeuronCores.

Sharding: data-parallel over batch — 32 batches per core, full QKV weights
replicated on every core, no collectives.

Per-core, per-batch dataflow (Tile framework pipelines across batches):
  1. Cast-DMA x[b] fp32->bf16 into SBUF, natural layout [T-chunk<=128, C].
  2. PE-transpose to x^T (C on partitions) via identity matmuls; evacuate
     PSUM->SBUF with copies split between ScalarE and VectorE.
  3. QKV projections with packed stationary [Wq|Wk] -> q^T,k^T in one pass;
     Wv -> v^T (+ ones row), PE-transposed back to v_ext=[v|1] per s-chunk.
  4. Scores wei^T[s-chunk] = k^T_slice.T @ q^T for the causal t-range only;
     exp on ScalarE with the 1/sqrt(C) scale fused; the below-diagonal
     triangle of the leading block is zeroed with gpsimd affine_select.
  5. out_ext[t-chunk] = sum_s p^T.T @ [v|1] accumulated in PSUM; the ones
     column yields the softmax denominator Z. VectorE reciprocal +
     per-partition tensor_scalar_mul normalizes; DMA out fp32.
"""

import numpy as np

import concourse.bass as bass  # noqa: F401  (bass types reachable via bacc/tile)
import concourse.mybir as mybir
import concourse.tile as tile
from concourse import bacc
from concourse.bass_utils import run_bass_kernel_spmd
from concourse.masks import make_identity

B, T, C, H = 256, 320, 1024, 64
NCORES = 8
BPC = B // NCORES  # batches per core
F32 = mybir.dt.float32
BF16 = mybir.dt.bfloat16
TCH = [(0, 128), (128, 128), (256, 64)]  # (offset, size) chunks of T
KC = C // 128  # 8 contraction chunks
SCALE = 1.0 / 32.0  # C**-0.5, folded into exp

_built = None


def _build():
    global _built
    if _built is not None:
        return _built

    nc = bacc.Bacc(
        "TRN2", target_bir_lowering=False, debug=False, enable_asserts=False
    )
    x_d = nc.dram_tensor("x", [BPC, T, C], F32, kind="ExternalInput").ap()
    wk_d = nc.dram_tensor("W_key", [C, H], F32, kind="ExternalInput").ap()
    wq_d = nc.dram_tensor("W_query", [C, H], F32, kind="ExternalInput").ap()
    wv_d = nc.dram_tensor("W_value", [C, H], F32, kind="ExternalInput").ap()
    out_d = nc.dram_tensor("out", [BPC, T, H], F32, kind="ExternalOutput").ap()

    EXP = mybir.ActivationFunctionType.Exp

    with tile.TileContext(nc) as tc:
        with (
            tc.tile_pool(name="consts", bufs=1) as consts,
            tc.tile_pool(name="xn", bufs=6) as xn_pool,
            tc.tile_pool(name="xt", bufs=2) as xt_pool,
            tc.tile_pool(name="qk", bufs=2) as qk_pool,
            tc.tile_pool(name="pp", bufs=6) as p_pool,
            tc.tile_pool(name="vt", bufs=2) as vt_pool,
            tc.tile_pool(name="vv", bufs=6) as v_pool,
            tc.tile_pool(name="rr", bufs=6) as r_pool,
            tc.tile_pool(name="ob", bufs=4) as o_pool,
            tc.tile_pool(name="pst", bufs=3, space="PSUM") as pst_pool,
            tc.tile_pool(name="psb", bufs=3, space="PSUM") as psb_pool,
            tc.tile_pool(name="pss", bufs=2, space="PSUM") as pss_pool,
        ):
            ident = consts.tile([128, 128], BF16, tag="ident")
            make_identity(nc, ident[:])
            # pass-1 stationary [Wv|Wq]: out rows 0:64 = v^T, 64:128 = q^T
            wvq = consts.tile([128, KC, 2 * H], BF16, tag="wvq")
            nc.gpsimd.dma_start(
                out=wvq[:, :, 0:H],
                in_=wv_d.rearrange("(kc p) h -> p kc h", p=128),
            )
            nc.gpsimd.dma_start(
                out=wvq[:, :, H : 2 * H],
                in_=wq_d.rearrange("(kc p) h -> p kc h", p=128),
            )
            # pass-2 stationary [Wk|Wk]: only out rows 64:128 (k^T) are used,
            # so k^T shares base partition 64 with q^T for the scores matmul
            wkk = consts.tile([128, KC, 2 * H], BF16, tag="wkk")
            nc.gpsimd.dma_start(
                out=wkk[:, :, 0:H],
                in_=wk_d.rearrange("(kc p) h -> p kc h", p=128),
            )
            nc.gpsimd.dma_start(
                out=wkk[:, :, H : 2 * H],
                in_=wk_d.rearrange("(kc p) h -> p kc h", p=128),
            )

            for b in range(BPC):
                # -- load x natural (cast to bf16 during DMA) --
                xns = []
                for toff, tsz in TCH:
                    xn = xn_pool.tile([128, C], BF16, tag="xn")
                    nc.gpsimd.dma_start(
                        out=xn[0:tsz, :], in_=x_d[b, toff : toff + tsz, :]
                    )
                    xns.append(xn)

                # -- transpose x -> x^T [C-chunk on partitions, T free] --
                xt = xt_pool.tile([128, KC, T], BF16, tag="xt")
                for kc in range(KC):
                    pst = pst_pool.tile([128, T], BF16, tag="pst")
                    for ti, (toff, tsz) in enumerate(TCH):
                        nc.tensor.transpose(
                            pst[:, toff : toff + tsz],
                            xns[ti][0:tsz, kc * 128 : (kc + 1) * 128],
                            ident[0:tsz, 0:tsz],
                        )
                    if kc < 4:
                        nc.scalar.copy(xt[:, kc, :], pst[:])
                    else:
                        nc.vector.tensor_copy(xt[:, kc, :], pst[:])

                # -- pass 1: [Wv|Wq].T @ x^T -> rows 0:64 v^T, rows 64:128 q^T --
                ps_1 = psb_pool.tile([128, T], F32, tag="big")
                for kc in range(KC):
                    nc.tensor.matmul(
                        ps_1[:],
                        wvq[:, kc, :],
                        xt[:, kc, :],
                        start=(kc == 0),
                        stop=(kc == KC - 1),
                    )
                q_sb = qk_pool.tile([128, T], BF16, tag="qq")
                nc.scalar.copy(q_sb[64:128, :], ps_1[64:128, :])
                vte = vt_pool.tile([65, T], BF16, tag="vt")
                nc.scalar.copy(vte[0:64, :], ps_1[0:64, :])
                nc.gpsimd.memset(vte[64:65, :], 1.0)

                # -- pass 2: [Wk|Wk].T @ x^T -> rows 64:128 k^T --
                ps_2 = psb_pool.tile([128, T], F32, tag="big")
                for kc in range(KC):
                    nc.tensor.matmul(
                        ps_2[:],
                        wkk[:, kc, :],
                        xt[:, kc, :],
                        start=(kc == 0),
                        stop=(kc == KC - 1),
                    )
                k_sb = qk_pool.tile([128, T], BF16, tag="kk")
                nc.scalar.copy(k_sb[64:128, :], ps_2[64:128, :])

                vs = []
                for soff, ssz in TCH:
                    ps_v = pss_pool.tile([128, 65], BF16, tag="small")
                    nc.tensor.transpose(
                        ps_v[0:ssz, :],
                        vte[0:65, soff : soff + ssz],
                        ident[0:65, 0:65],
                    )
                    v_sb = v_pool.tile([128, 65], BF16, tag="vv")
                    nc.vector.tensor_copy(v_sb[0:ssz, :], ps_v[0:ssz, :])
                    vs.append(v_sb)

                # -- scores (causal t-range only) + exp --
                ps = []
                for soff, ssz in TCH:
                    nt = T - soff
                    ps_w = psb_pool.tile([128, T], F32, tag="big")
                    nc.tensor.matmul(
                        ps_w[0:ssz, 0:nt],
                        k_sb[64:128, soff : soff + ssz],
                        q_sb[64:128, soff:T],
                        start=True,
                        stop=True,
                    )
                    p_sb = p_pool.tile([128, T], BF16, tag="pp")
                    nc.scalar.activation(
                        p_sb[0:ssz, 0:nt], ps_w[0:ssz, 0:nt], EXP, scale=SCALE
                    )
                    # zero strictly-below-diagonal of the leading [ssz,ssz] block:
                    # keep where (-s_loc + t_loc) >= 0
                    nc.gpsimd.affine_select(
                        out=p_sb[0:ssz, 0:ssz],
                        in_=p_sb[0:ssz, 0:ssz],
                        compare_op=mybir.AluOpType.is_ge,
                        fill=0.0,
                        base=0,
                        pattern=[[1, ssz]],
                        channel_multiplier=-1,
                    )
                    ps.append(p_sb)

                # -- out_ext = sum_s p^T.T @ [v|1]; normalize by Z; store --
                for ti, (toff, tsz) in enumerate(TCH):
                    ps_o = pss_pool.tile([128, 65], F32, tag="small")
                    for si in range(ti + 1):
                        soff, ssz = TCH[si]
                        col = toff - soff
                        nc.tensor.matmul(
                            ps_o[0:tsz, :],
                            ps[si][0:ssz, col : col + tsz],
                            vs[si][0:ssz, :],
                            start=(si == 0),
                            stop=(si == ti),
                        )
                    r = r_pool.tile([128, 1], F32, tag="rr")
                    nc.vector.reciprocal(r[0:tsz, :], ps_o[0:tsz, 64:65])
                    o_sb = o_pool.tile([128, H], F32, tag="ob")
                    nc.vector.tensor_scalar_mul(
                        o_sb[0:tsz, :], ps_o[0:tsz, 0:H], r[0:tsz, :]
                    )
                    nc.sync.dma_start(
                        out=out_d[b, toff : toff + tsz, :], in_=o_sb[0:tsz, :]
                    )

    nc.compile()
    _built = nc
    return nc


def _run(inputs, trace=False, trace_cores=None):
    nc = _build()
    x = np.ascontiguousarray(np.asarray(inputs["x"], dtype=np.float32))
    wk = np.ascontiguousarray(np.asarray(inputs["W_key"], dtype=np.float32))
    wq = np.ascontiguousarray(np.asarray(inputs["W_query"], dtype=np.float32))
    wv = np.ascontiguousarray(np.asarray(inputs["W_value"], dtype=np.float32))
    in_maps = [
        {
            "x": np.ascontiguousarray(x[i * BPC : (i + 1) * BPC]),
            "W_key": wk,
            "W_query": wq,
            "W_value": wv,
        }
        for i in range(NCORES)
    ]
    res = run_bass_kernel_spmd(
        nc,
        in_maps,
        core_ids=list(range(NCORES)),
        trace=trace,
        trace_cores=trace_cores,
    )
    out = np.concatenate([r["out"] for r in res.results], axis=0)
    return out, res


def kernel(x, W_key, W_query, W_value):
    out, _ = _run(
        {"x": x, "W_key": W_key, "W_query": W_query, "W_value": W_value}
    )
    return out
